# revision 29
# baseline (speedup 1.0000x reference)
import sys

for _p in ("/opt/trn_rl_repo",):
    if _p not in sys.path:
        sys.path.insert(0, _p)

import numpy as np

B, G, DIM, N = 4, 512, 384, 25088
IMAGE = 224
KS = 8
POOL = IMAGE // KS            # 28
NCORES = 8
HALF = N // 2                 # 12544 points per core
BANDS = 7                     # pool rows per core (56 image rows / 8)
TPB = 14                      # tiles per band
PPT = 128                     # points per tile
BAND_PTS = TPB * PPT          # 1792 = 8 image rows
HPTS = BAND_PTS // 2          # 896 = half-band points
NU = BANDS * 2                # 14 half-band pipeline units
KA = 24                       # augmented bf16 contraction rows

_CACHE = {}


def _build_program():
    import concourse.mybir as mybir
    from concourse.bacc import Bacc
    from concourse.tile import TileContext
    from concourse.alu_op_type import AluOpType

    f32 = mybir.dt.float32
    f16 = mybir.dt.float16
    bf16 = mybir.dt.bfloat16
    u16 = mybir.dt.uint16
    i16 = mybir.dt.int16

    nc = Bacc()

    ptsA_d = nc.dram_tensor("ptsA", [KA, HALF], bf16, kind="ExternalInput")
    cenA_d = nc.dram_tensor("cenA", [KA, G], bf16, kind="ExternalInput")
    feat_d = nc.dram_tensor("featp", [128, 4, DIM], f32, kind="ExternalInput")
    ssel_d = nc.dram_tensor("ssel", [128, 7, POOL], f16, kind="ExternalInput")
    eye_d = nc.dram_tensor("eye28", [POOL, POOL], f32, kind="ExternalInput")
    # [partition, dim-chunk, pool-col]: one band's 3 dim-chunks go out in a
    # single DMA; host reshapes (128,3,196) -> (384,196)
    out_d = nc.dram_tensor("out", [128, 3, BANDS * POOL], f32, kind="ExternalOutput")

    with TileContext(nc) as tc:
        with tc.sbuf_pool(name="const", bufs=1) as cpool, \
             tc.sbuf_pool(name="bandio", bufs=3) as bpool, \
             tc.sbuf_pool(name="sel", bufs=2) as spool, \
             tc.sbuf_pool(name="tile", bufs=4) as tpool, \
             tc.sbuf_pool(name="wpool", bufs=3) as wpool, \
             tc.sbuf_pool(name="accout", bufs=1) as apool, \
             tc.sbuf_pool(name="ostage", bufs=3) as opool, \
             tc.psum_pool(name="ps_s", bufs=2) as ps_s_pool, \
             tc.psum_pool(name="ps_a", bufs=1) as ps_a_pool, \
             tc.psum_pool(name="ps_t", bufs=2) as ps_t_pool, \
             tc.psum_pool(name="ps_o", bufs=1) as ps_o_pool:

            # trigger from ACT so it runs concurrently with SP's ptsA_b0 load
            cenA = cpool.tile([KA, G], bf16, name="cenA_sb")
            nc.scalar.dma_start(out=cenA, in_=cenA_d[:])
            ssel = cpool.tile([128, 7, POOL], f16, name="ssel_sb")
            feats = cpool.tile([128, 4, DIM], f32, name="feat_sb")
            eye = cpool.tile([POOL, POOL], f32, name="eye_sb")
            atsb = apool.tile([128, 4, BANDS, POOL], f32, name="atsb")

            # software pipeline at half-band granularity: scatter/accum for
            # unit u-1, then selection for unit u
            sel_state = {}   # u -> list of (w4, i4, tt0, cnt) weight chunks
            band_vi = {}
            band_aps = {}

            def emit_weights(u, tt0, cnt):
                """d2 = max(-sprime_top3, 1e-10) -> normalized inv-dist w4/i4
                for tiles [tt0, tt0+cnt) of unit u."""
                bd, hb = u // 2, u % 2
                vband, iband = band_vi[bd]
                t0 = hb * 7 + tt0
                v3 = vband[:, t0:t0 + cnt, 0:3]
                d2 = spool.tile([128, cnt, 3], f32, name=f"d2{u}_{tt0}", tag=f"d2{tt0}")
                nc.gpsimd.tensor_scalar(
                    out=d2,
                    in0=v3,
                    scalar1=-1.0,
                    scalar2=1e-10,
                    op0=AluOpType.mult,
                    op1=AluOpType.max,
                )
                # inverse-distance weights without reciprocals (keeps DVE
                # free): w_k = (d_i*d_j) / (d_1*d_2 + d_0*d_2 + d_0*d_1)
                m = spool.tile([128, cnt, 3], f32, name=f"m{u}_{tt0}", tag=f"m{tt0}")
                nc.gpsimd.tensor_tensor(
                    out=m[:, :, 0], in0=d2[:, :, 1], in1=d2[:, :, 2], op=AluOpType.mult
                )
                nc.gpsimd.tensor_tensor(
                    out=m[:, :, 1], in0=d2[:, :, 0], in1=d2[:, :, 2], op=AluOpType.mult
                )
                nc.gpsimd.tensor_tensor(
                    out=m[:, :, 2], in0=d2[:, :, 0], in1=d2[:, :, 1], op=AluOpType.mult
                )
                dn = spool.tile([128, cnt, 2], f32, name=f"dn{u}_{tt0}", tag=f"dn{tt0}")
                nc.gpsimd.tensor_tensor(
                    out=dn[:, :, 1], in0=m[:, :, 0], in1=m[:, :, 1], op=AluOpType.add
                )
                nc.gpsimd.tensor_tensor(
                    out=dn[:, :, 0], in0=dn[:, :, 1], in1=m[:, :, 2], op=AluOpType.add
                )
                w4 = spool.tile([128, cnt, 4], f16, name=f"w4{u}_{tt0}", tag=f"w4{tt0}")
                nc.gpsimd.memset(w4, 0)
                nc.gpsimd.tensor_tensor(
                    out=w4[:, :, 0:3],
                    in0=m,
                    in1=dn[:, :, 0:1].broadcast_to([128, cnt, 3]),
                    op=AluOpType.divide,
                )
                i4 = spool.tile([128, cnt, 4], i16, name=f"i4{u}_{tt0}", tag=f"i4{tt0}")
                nc.gpsimd.memset(i4, -1)
                nc.gpsimd.tensor_copy(
                    out=i4[:, :, 0:3], in_=iband[:, t0:t0 + cnt, 0:3].bitcast(i16)
                )
                sel_state.setdefault(u, []).append((w4, i4, tt0, cnt))

            def emit_scatter_chunk(u, w4, i4, tt0, cnt):
                bd, hb = u // 2, u % 2
                a_ps = band_aps[bd]
                for i in range(cnt):
                    tt = tt0 + i
                    wt = wpool.tile([128, G], f16, name=f"wt{u}_{tt}", tag="wt")
                    nc.gpsimd.local_scatter(
                        out_ap=wt,
                        data_ap=w4[:, i, :],
                        idxs_ap=i4[:, i, :],
                        channels=128,
                        num_elems=G,
                        num_idxs=4,
                    )
                    nc.tensor.matmul(
                        out=a_ps,
                        lhsT=ssel[:, tt, :],
                        rhs=wt,
                        start=(hb == 0 and tt == 0),
                        stop=(hb == 1 and tt == 6),
                    )

            def emit_band_out(bd):
                """a_ps -> transpose -> finals -> incremental output DMA."""
                a_ps = band_aps.pop(bd)
                tail = bd == BANDS - 1
                acp = spool.tile([POOL, G], f32, name=f"acp{bd}", tag="acp")
                if tail:
                    # split the drain-tail copy across ACT and the idle DVE
                    nc.scalar.copy(out=acp[:, 0:256], in_=a_ps[:, 0:256])
                    nc.vector.tensor_copy(out=acp[:, 256:512], in_=a_ps[:, 256:512])
                else:
                    nc.scalar.copy(out=acp, in_=a_ps)
                # all 4 transposes land in one PSUM tile (448B, one bank) so
                # a single strided copy moves them to SBUF
                t_ps = ps_t_pool.tile([128, 4, POOL], f32, name=f"t_ps{bd}", tag="t_ps")
                for c in range(4):
                    nc.tensor.transpose(
                        out=t_ps[:, c, :], in_=acp[:, c * 128:(c + 1) * 128], identity=eye
                    )
                if tail:
                    nc.scalar.copy(out=atsb[:, 0:2, bd, :], in_=t_ps[:, 0:2, :])
                    nc.vector.tensor_copy(out=atsb[:, 2:4, bd, :], in_=t_ps[:, 2:4, :])
                else:
                    nc.scalar.copy(out=atsb[:, :, bd, :], in_=t_ps)
                # fold finals per band: pooled[d, bd*28+pc] = sum_g feat[g,d]*AT[g,bd,pc]
                for dc in range(3):
                    for gc in range(4):
                        nc.tensor.matmul(
                            out=o_ps[dc][:, bd * POOL:(bd + 1) * POOL],
                            lhsT=feats[:, gc, dc * 128:(dc + 1) * 128],
                            rhs=atsb[:, gc, bd, :],
                            start=(gc == 0),
                            stop=(gc == 3),
                        )
                osb = opool.tile([128, 3, POOL], f32, name=f"osb{bd}", tag="osb")
                for dc in range(3):
                    if tail and dc % 2 == 1:
                        nc.vector.tensor_copy(
                            out=osb[:, dc, :],
                            in_=o_ps[dc][:, bd * POOL:(bd + 1) * POOL],
                        )
                    else:
                        nc.scalar.copy(
                            out=osb[:, dc, :], in_=o_ps[dc][:, bd * POOL:(bd + 1) * POOL]
                        )
                nc.sync.dma_start(
                    out=out_d[:, :, bd * POOL:(bd + 1) * POOL], in_=osb
                )

            def emit_scatter_half(u):
                bd, hb = u // 2, u % 2
                if hb == 0:
                    band_aps[bd] = ps_a_pool.tile(
                        [POOL, G], f32, name=f"a_ps{bd}", tag="a_ps"
                    )
                for w4, i4, tt0, cnt in sel_state.pop(u):
                    emit_scatter_chunk(u, w4, i4, tt0, cnt)
                if hb == 1:
                    emit_band_out(bd)

            def emit_sel_half(u):
                bd, hb = u // 2, u % 2
                ptsA_b = bpool.tile([KA, HPTS], bf16, name=f"ptsA_b{u}", tag="ptsA_b")
                nc.sync.dma_start(
                    out=ptsA_b, in_=ptsA_d[:, u * HPTS:(u + 1) * HPTS]
                )
                if hb == 0:
                    vband = spool.tile([128, TPB, 8], f32, name=f"vband{bd}", tag="vband")
                    iband = spool.tile([128, TPB, 8], u16, name=f"iband{bd}", tag="iband")
                    band_vi[bd] = (vband, iband)
                else:
                    vband, iband = band_vi[bd]

                last = u == NU - 1
                for tt in range(7):
                    t = hb * 7 + tt
                    # PSUM = -d2 = 2p.c - pn - cn in ONE bf16 matmul: each
                    # fp32 factor is a 3-term bf16 split (h+l+l2), every kept
                    # cross product is exact in fp32 PSUM, dropped terms are
                    # <= 2^-27 relative. K=24 rows cost the same as K=3.
                    s_ps = ps_s_pool.tile([128, G], f32, name=f"s_ps{u}_{tt}", tag="s_ps")
                    nc.tensor.matmul(
                        out=s_ps,
                        lhsT=ptsA_b[:, tt * PPT:(tt + 1) * PPT],
                        rhs=cenA,
                        start=True,
                        stop=True,
                    )
                    ssb = tpool.tile([128, G], f32, name=f"ssb{u}_{tt}", tag="ssb")
                    if u == 0 and tt == 0:
                        # very first tile: max straight from PSUM (concurrent
                        # with the ACT copy) so DVE starts ~700ns earlier; the
                        # index pass reads the bit-identical SBUF copy so the
                        # PSUM bank frees early
                        nc.vector.max(out=vband[:, t, :], in_=s_ps)
                        nc.scalar.copy(out=ssb, in_=s_ps)
                    else:
                        nc.scalar.copy(out=ssb, in_=s_ps)
                        nc.vector.max(out=vband[:, t, :], in_=ssb)
                    nc.vector.max_index(
                        out=iband[:, t, :], in_max=vband[:, t, :], in_values=ssb
                    )
                    if last and tt == 4:
                        # drain the final unit early: scatter tiles 0-4 while
                        # tiles 5-6 are still selecting
                        emit_weights(u, 0, 5)
                        for chunk in sel_state.pop(u):
                            emit_scatter_chunk(u, *chunk)
                if last:
                    emit_weights(u, 5, 2)
                else:
                    emit_weights(u, 0, 7)

            o_ps = [
                ps_o_pool.tile([128, BANDS * POOL], f32, name=f"o_ps{dc}", tag=f"o_ps{dc}")
                for dc in range(3)
            ]
            for u in range(NU + 1):
                if u == 1:
                    # deferred const loads: needed first by scatter (ssel/eye)
                    # and finals (feats) — keep them off unit-0's critical path
                    nc.sync.dma_start(out=ssel, in_=ssel_d[:])
                    nc.sync.dma_start(out=eye, in_=eye_d[:])
                    nc.sync.dma_start(out=feats, in_=feat_d[:])
                if u >= 1:
                    emit_scatter_half(u - 1)
                if u < NU:
                    emit_sel_half(u)

    nc.finalize()
    return nc


def _split3(v32):
    """3-term bf16 split of an fp32 array: v ~= h + l + l2 to ~2^-27 rel."""
    import ml_dtypes

    bf = ml_dtypes.bfloat16
    v = v32.astype(np.float64)
    h = v32.astype(bf)
    l = (v - h.astype(np.float64)).astype(np.float32).astype(bf)
    l2 = (v - h.astype(np.float64) - l.astype(np.float64)).astype(np.float32).astype(bf)
    return h, l, l2


def _aug24(coords2, norm):
    """Build the 24 bf16 rows for one side of the -d2 matmul.

    coords2: (3, M) fp32 — the 2x/2y/2z (points) or cx/cy/cz (centers) rows.
    norm: (M,) fp32 — pn (points) or cn (centers); negated chunks go in the
    rows that multiply the other side's constant-1 rows.

    Row pairing (point row k) x (center row k), small terms first:
      k=0..8   : per-coord (h,l2), (l,l), (l2,h)
      k=9,10   : (-pn3, 1), (1, -cn3)
      k=11..16 : per-coord (l,h), (h,l)
      k=17,18  : (-pn2, 1), (1, -cn2)
      k=19..21 : per-coord (h,h)
      k=22,23  : (-pn1, 1), (1, -cn1)
    `which` selects the point-side or center-side entry of each pair.
    """
    import ml_dtypes

    bf = ml_dtypes.bfloat16
    M = coords2.shape[1]
    h, l, l2 = _split3(coords2)          # each (3, M)
    n1, n2, n3 = _split3(norm)           # each (M,)
    ones = np.ones(M, dtype=bf)
    return h, l, l2, (-n1.astype(np.float32)).astype(bf), (
        -n2.astype(np.float32)
    ).astype(bf), (-n3.astype(np.float32)).astype(bf), ones


def _pack24(p_parts, c_parts):
    """Assemble matched (24, Mp) and (24, Mc) bf16 row stacks."""
    ph, pl, pl2, pn1, pn2, pn3, pones = p_parts
    ch, cl, cl2, cn1, cn2, cn3, cones = c_parts
    prows, crows = [], []

    def add(pr, cr):
        prows.append(pr)
        crows.append(cr)

    for c in range(3):
        add(ph[c], cl2[c])
        add(pl[c], cl[c])
        add(pl2[c], ch[c])
    add(pn3, cones)
    add(pones, cn3)
    for c in range(3):
        add(pl[c], ch[c])
        add(ph[c], cl[c])
    add(pn2, cones)
    add(pones, cn2)
    for c in range(3):
        add(ph[c], ch[c])
    add(pn1, cones)
    add(pones, cn1)
    return (
        np.ascontiguousarray(np.stack(prows)),
        np.ascontiguousarray(np.stack(crows)),
    )


def _host_inputs(group_features, group_centers, original_points, core):
    b, h = core // 2, core % 2
    pts = np.asarray(original_points[b, h * HALF:(h + 1) * HALF], dtype=np.float32)
    x, y, z = pts[:, 0].copy(), pts[:, 1].copy(), pts[:, 2].copy()
    # pn in device add order: (x^2 + y^2) + z^2, fp32
    pn = (x * x + y * y) + z * z
    p_parts = _aug24(np.stack([2.0 * x, 2.0 * y, 2.0 * z]), pn)

    cen = np.asarray(group_centers[b], dtype=np.float32)   # (512, 3)
    cx, cy, cz = cen[:, 0].copy(), cen[:, 1].copy(), cen[:, 2].copy()
    cn = (cx * cx + cy * cy) + cz * cz
    c_parts = _aug24(np.stack([cx, cy, cz]), cn)

    ptsA, cenA = _pack24(p_parts, c_parts)

    feat = np.asarray(group_features[b], dtype=np.float32)  # (512, 384)
    featp = np.ascontiguousarray(feat.reshape(4, 128, DIM).transpose(1, 0, 2))

    return {
        "ptsA": ptsA,
        "cenA": cenA,
        "featp": featp,
        "ssel": _ssel(),
        "eye28": np.eye(POOL, dtype=np.float32),
    }


def _ssel():
    s = np.zeros((128, 7, POOL), dtype=np.float16)
    for phi in range(7):
        for p in range(128):
            pc = ((phi * 128 + p) % IMAGE) // KS
            s[p, phi, pc] = 1.0 / 64.0
    return s


def _numpy_fallback(group_features, group_centers, original_points, nonzero_indices, kernel_size):
    gf = np.asarray(group_features, dtype=np.float64)
    cen = np.asarray(group_centers, dtype=np.float64)
    pts = np.asarray(original_points, dtype=np.float64)
    ks = int(kernel_size)
    out = np.zeros((B, DIM, IMAGE * IMAGE), dtype=np.float64)
    for b in range(B):
        d2 = (
            np.sum(pts[b] ** 2, axis=1)[:, None]
            + np.sum(cen[b] ** 2, axis=1)[None, :]
            - 2.0 * pts[b] @ cen[b].T
        )
        idx = np.argsort(d2, axis=1)[:, :3]
        d = np.maximum(np.take_along_axis(d2, idx, axis=1), 1e-10)
        rec = 1.0 / d
        w = rec / rec.sum(axis=1, keepdims=True)
        interp = np.einsum("nkd,nk->dn", gf[b][idx], w)
        out[b][:, np.asarray(nonzero_indices)] = interp
    ho = IMAGE // ks
    pooled = out.reshape(B, DIM, ho, ks, ho, ks).mean(axis=(3, 5))
    return pooled.astype(np.float32)


def kernel(group_features, group_centers, original_points, nonzero_indices, kernel_size):
    nz = np.asarray(nonzero_indices)
    ks = int(np.asarray(kernel_size))
    if ks != KS or nz.shape != (N,) or not np.array_equal(nz, np.arange(N)):
        return _numpy_fallback(
            group_features, group_centers, original_points, nonzero_indices, kernel_size
        )

    from concourse.bass_utils import run_bass_kernel_spmd

    if "nc" not in _CACHE:
        _CACHE["nc"] = _build_program()
    nc = _CACHE["nc"]

    in_maps = [
        _host_inputs(group_features, group_centers, original_points, c)
        for c in range(NCORES)
    ]
    res = run_bass_kernel_spmd(nc, in_maps, core_ids=list(range(NCORES))).results

    out = np.zeros((B, DIM, POOL, POOL), dtype=np.float32)
    for c in range(NCORES):
        b, h = c // 2, c % 2
        o = np.asarray(res[c]["out"]).reshape(128, 3, BANDS * POOL)
        o = o.transpose(1, 0, 2).reshape(DIM, BANDS, POOL)
        out[b, :, 7 * h:7 * h + 7, :] = o
    return out
